# revision 42
# baseline (speedup 1.0000x reference)
"""CapsNet routing layer (nn_CapsLayer) on 8 Trainium2 NeuronCores.

reference:
    u_hat = einsum("ncoi,bci->bnco", W[0], x)         # B,N,C,O = 1024,2,512,64
    3 dynamic-routing iterations (softmax over n, weighted sum over c,
    squash, agreement update); returns v from iteration 3.

Strategy (in-caps sharded, hardcoded shapes):
  - 8 cores x 64 in-caps, every core sees the full batch. Per-core DMA
    is 32 MiB of x (fp16) + 4 MiB of W (fp16, resident in SBUF).
  - GEMM is a single fp16 x fp16 pass (fp32 PSUM accumulate): ~3e-4
    relative accuracy at full PE rate. While the GEMM accumulates u per
    in-cap, a second PSUM tile accumulates sum_c u (iteration-0's s) via
    the same stationary tiles -- iteration 0's reduction costs zero DVE.
  - The batch is 8 pipelined chunks of 128 samples; each chunk's u_hat
    is stored fp16 in SBUF twice: layout A (b, n, c, o) for the
    agreement pass and layout B (b, n, o, c) for the weighted-sum pass,
    so every DVE op has a packed 2-byte innermost axis -> 2x DVE mode.
  - Routing reductions are pairwise TT-add trees (2x mode, 0.5 cyc/elem)
    instead of TensorReduce (1 cyc/elem, no fast mode). Multiplies are
    TT with the small operand broadcast on a middle axis (innermost
    stays packed). Squash/sigmoid/d stay fp32 (they are tiny).
  - The per-iteration cross-core reduction of s is batched per chunk
    PAIR: one 128 KB AllReduce per (pair, iteration) = 12 total,
    overlapped with the partner chunk's routing work.
"""
import os
import sys
import types

sys.path.insert(0, "/opt/trn_rl_repo")

import numpy as np
import concourse.bass as bass
import concourse.mybir as mybir
import concourse.tile as tile
from concourse.bass_utils import run_bass_kernel_spmd

B, NCAPS, C, ICH, OCH = 1024, 2, 512, 256, 64
ITERATIONS = 3
NCORES = 8
CPC = C // NCORES            # in-caps per core = 64
NBCH = 8                     # batch chunks
BCH = B // NBCH              # samples per chunk = 128
KH = 2                       # K tiles (ICH = 2*128)
CG = 8                       # c's per GEMM/DMA group
NO = NCAPS * OCH             # 128

FP32 = mybir.dt.float32
FP16 = mybir.dt.float16
ADD = mybir.AluOpType.add
MULT = mybir.AluOpType.mult
SUB = mybir.AluOpType.subtract
AF = mybir.ActivationFunctionType
AX = mybir.AxisListType

LAST_EXEC_NS = None


def _install_profile_hook():
    """antenv.axon_hooks is absent in this image; recreate it so
    run_bass_kernel_spmd(trace=True)/BASS_TRACE can report exec_time_ns."""
    if "antenv.axon_hooks" in sys.modules:
        return
    mod = types.ModuleType("antenv.axon_hooks")
    mod._hook = None
    mod.set_axon_ntff_profile_hook = lambda h: setattr(mod, "_hook", h)
    mod.get_axon_ntff_profile_hook = lambda: mod._hook
    sys.modules["antenv.axon_hooks"] = mod
    try:
        from trn_agent_boot.trn_boot import _ntff_profile_via_ctypes

        hook = _ntff_profile_via_ctypes("/opt/axon/libaxon_pjrt.so")
        if hook is not None:
            mod._hook = hook
    except Exception:
        pass


def _split_sync_waits(nc, max_waits=1):
    """walrus setupSyncWait rejects instructions with more than one sem
    wait; hoist extras onto same-engine InstNoOp's placed just before."""
    for f in nc.m.functions:
        for bb in f.blocks:
            out = []
            changed = False
            for inst in bb.instructions:
                si = inst.sync_info
                waits = list(si.on_wait) if si is not None and si.on_wait else []
                if len(waits) > max_waits:
                    extra, keep = waits[:-max_waits], waits[-max_waits:]
                    for g, w in enumerate(extra):
                        out.append(
                            mybir.InstNoOp(
                                name=f"{inst.name}_wsplit{g}",
                                engine=inst.engine,
                                bass_nofuse=True,
                                sync_info=mybir.SyncInfo(on_wait=[w], on_update=[]),
                            )
                        )
                    inst.sync_info = mybir.SyncInfo(
                        on_wait=keep,
                        on_update=list(si.on_update) if si.on_update else [],
                    )
                    changed = True
                out.append(inst)
            if changed:
                bb.instructions = out


def build_kernel(split_waits=True):
    nc = bass.Bass(
        "TRN2", target_bir_lowering=False, debug=False, num_devices=NCORES
    )
    # x shard: [h, i, bchunk, c, b] fp16
    xt = nc.dram_tensor("xt", [KH, 128, NBCH, CPC, BCH], FP16, kind="ExternalInput").ap()
    # W shard: [h, i, c, (n,o)] fp16
    wt = nc.dram_tensor("wt", [KH, 128, CPC, NO], FP16, kind="ExternalInput").ap()
    out = nc.dram_tensor("out", [B, NCAPS, OCH], FP32, kind="ExternalOutput").ap()

    with tile.TileContext(nc) as tc:
        with (
            tc.tile_pool(name="xin", bufs=3) as xpool,
            tc.tile_pool(name="psg", bufs=3, space="PSUM") as pgpool,
            tc.tile_pool(name="ps0", bufs=2, space="PSUM") as p0pool,
            tc.tile_pool(name="ua", bufs=3) as uapool,
            tc.tile_pool(name="ub", bufs=3) as ubpool,
            tc.tile_pool(name="dram", bufs=4, space="DRAM") as drpool,
        ):
            # resident W: per-h tiles (128i, c*no) fp16
            wsb = {}
            for h in range(KH):
                t = nc.alloc_sbuf_tensor(f"w{h}", [128, CPC * NO], FP16).ap()
                nc.sync.dma_start(t[:], wt[h].rearrange("i c f -> i (c f)"))
                wsb[h] = t

            # routing tensors, duplicated by chunk parity so the pair's
            # chunks can interleave
            d_all = nc.alloc_sbuf_tensor("d_all", [128, NBCH, CPC], FP32).ap()
            P = 2
            # product / tree scratch (fp16): flat, with contiguous views as
            # layout A (b,n,c,o) for the y pass and layout B (b,n,o,c) for
            # the s pass
            wflat = [nc.alloc_sbuf_tensor(f"wscr{p}", [128, NCAPS * CPC * OCH], FP16).ap() for p in range(P)]
            wA_v = [w.rearrange("b (n c o) -> b n c o", n=NCAPS, c=CPC) for w in wflat]
            wB_v = [w.rearrange("b (n o c) -> b n o c", n=NCAPS, o=OCH) for w in wflat]
            tscr = [nc.alloc_sbuf_tensor(f"tscr{p}", [128, NCAPS, CPC, 32], FP16).ap() for p in range(P)]
            # collective staging in fp16 (halves AllReduce payload), per
            # chunk parity (sp0 for the GEMM-time iteration-0 partial so it
            # never WAR-blocks on sp)
            sp = [nc.alloc_sbuf_tensor(f"sp{p}", [128, NO], FP16).ap() for p in range(P)]
            sp0 = [nc.alloc_sbuf_tensor(f"sp0{p}", [128, NO], FP16).ap() for p in range(P)]
            # pair AllReduce results, double buffered by pair parity
            ssr = [nc.alloc_sbuf_tensor(f"ssr{r}", [128, P, NO], FP16).ap() for r in range(2)]
            # collective DRAM staging (Shared output enables the fast
            # direct-write collective path)
            bi_t = [nc.dram_tensor(f"ccin{r}", [128, P, NO], FP16, kind="Internal").ap() for r in range(2)]
            bo_t = [nc.dram_tensor(f"ccout{r}", [128, P, NO], FP16, kind="Internal", addr_space="Shared").ap() for r in range(2)]
            # small fp32 routing tensors, batched per PAIR (one squash chain
            # serves both chunks -> half the serial instruction count)
            ssP = nc.alloc_sbuf_tensor("ssP", [128, P, NCAPS, OCH], FP32).ap()
            s2P = nc.alloc_sbuf_tensor("s2P", [128, P, NCAPS, OCH], FP32).ap()
            sqP = nc.alloc_sbuf_tensor("sqP", [128, P, NCAPS], FP32).ap()
            rrP = nc.alloc_sbuf_tensor("rrP", [128, P, NCAPS], FP32).ap()
            denP = nc.alloc_sbuf_tensor("denP", [128, P, NCAPS], FP32).ap()
            recP = nc.alloc_sbuf_tensor("recP", [128, P, NCAPS], FP32).ap()
            mmvP = nc.alloc_sbuf_tensor("mmvP", [128, P, NCAPS], FP32).ap()
            v16P = nc.alloc_sbuf_tensor("v16P", [128, P, NCAPS, OCH], FP16).ap()
            v32P = nc.alloc_sbuf_tensor("v32P", [128, P, NCAPS, OCH], FP32).ap()
            yy = [nc.alloc_sbuf_tensor(f"yy{p}", [128, NCAPS, CPC], FP32).ap() for p in range(P)]
            dd = [nc.alloc_sbuf_tensor(f"dd{p}", [128, CPC], FP32).ap() for p in range(P)]
            co16 = [nc.alloc_sbuf_tensor(f"co16{p}", [128, NCAPS, CPC], FP16).ap() for p in range(P)]

            def cc_pair(q, srcs):
                """pair-batched AllReduce of two staged fp16 partials ->
                ssr[q]."""
                # collective-adjacent DMAs ride the Vector queue: they wait
                # on DVE-produced data (or gate DVE-consumed data) the DVE
                # would stall for anyway, and a CC-blocked DMA at the head of
                # the Sync queue would starve the x-tile prefetch behind it
                bi = bi_t[q]
                bo = bo_t[q]
                for p in range(P):
                    nc.scalar.dma_start(bi[:, p, :], srcs[p])
                nc.gpsimd.collective_compute(
                    "AllReduce",
                    ADD,
                    replica_groups=[list(range(NCORES))],
                    ins=[bi[:].rearrange("b p f -> b (p f)").opt()],
                    outs=[bo[:].rearrange("b p f -> b (p f)").opt()],
                )
                nc.gpsimd.dma_start(ssr[q][:], bo[:])

            def gemm_chunk(bk):
                """generator: emits one cg-group per yield; finishes with the
                it-0 partial-s staging + this chunk's first AllReduce."""
                p = bk % 2
                uA = uapool.tile([128, NCAPS, CPC, OCH], FP16, tag="uA")
                uB = ubpool.tile([128, NCAPS, OCH, CPC], FP16, tag="uB")
                ps0 = p0pool.tile([BCH, NO], FP32, tag="ps0")
                us[bk] = (uA, uB)
                nmm = CPC * KH  # matmuls in the ps0 accumulation group
                mi = 0
                for cg in range(CPC // CG):
                    c0 = cg * CG
                    xth = {}
                    for h in range(KH):
                        t = xpool.tile([128, CG, BCH], FP16, tag=f"x{h}")
                        nc.sync.dma_start(t[:], xt[h, :, bk, c0 : c0 + CG, :])
                        xth[h] = t
                    pg = pgpool.tile([BCH, CG, NO], FP32, tag="pg")
                    for j in range(CG):
                        c = c0 + j
                        for h in range(KH):
                            wslice = wsb[h][:, c * NO : (c + 1) * NO]
                            nc.tensor.matmul(
                                pg[:, j, :],
                                lhsT=xth[h][:, j, :],
                                rhs=wslice,
                                start=(h == 0),
                                stop=(h == KH - 1),
                            )
                            # iteration-0 s: accumulate sum_c u on PE
                            nc.tensor.matmul(
                                ps0[:],
                                lhsT=xth[h][:, j, :],
                                rhs=wslice,
                                start=(mi == 0),
                                stop=(mi == nmm - 1),
                            )
                            mi += 1
                    # PSUM (b, (c,n,o)) -> uA (b, (n,c,o)) fp16
                    nc.scalar.copy(
                        uA[:, :, c0 : c0 + CG, :],
                        pg[:].rearrange("b c (n o) -> b n c o", n=NCAPS),
                    )
                    # PSUM (b, (c,n,o)) -> uB (b, (n,o,c)) fp16
                    nc.scalar.copy(
                        uB[:, :, :, c0 : c0 + CG],
                        pg[:].rearrange("b c (n o) -> b n o c", n=NCAPS),
                    )
                    yield
                # stage this chunk's it-0 partial s (DMA cannot read PSUM:
                # bounce through SBUF; fp16 halves the collective payload)
                nc.scalar.copy(sp0[p][:], ps0[:])
                yield

            def tree_reduce(p, src, inner, out_fp32):
                """pairwise-tree sum over the innermost axis of src
                ([128, NCAPS, M, inner] fp16), result -> out_fp32
                ([128, NCAPS, M] fp32). Ping-pongs between src and tscr."""
                ts = tscr[p]
                cur = src
                width = inner
                use_t = True
                while width > 2:
                    h = width // 2
                    dst = (ts if use_t else src)[:, :, :, 0:h]
                    nc.vector.tensor_tensor(
                        dst, cur[:, :, :, 0:h], cur[:, :, :, h:width], op=ADD
                    )
                    cur = dst
                    width = h
                    use_t = not use_t
                nc.vector.tensor_tensor(
                    out_fp32, cur[:, :, :, 0], cur[:, :, :, 1], op=ADD
                )

            def squash_pair(q, it):
                """ssr[q] (fp16 AllReduced s, both chunks) -> v16P (+v32P on
                the last iteration). it==0 applies the 0.5 softmax coeff."""
                s_in = ssr[q][:].rearrange("b p (n o) -> b p n o", n=NCAPS)
                if it == 0:
                    # iteration-0 softmax coeff is exactly 0.5
                    nc.vector.tensor_scalar(ssP[:], s_in, 0.5, None, op0=MULT)
                    sx = ssP[:]
                else:
                    # read the AllReduce result (fp16) directly -- a copy to
                    # fp32 here would sit on the post-collective critical path
                    sx = s_in
                nc.vector.tensor_tensor(s2P[:], sx, sx, op=MULT)
                nc.vector.tensor_reduce(
                    sqP[:].unsqueeze(3), s2P[:], axis=AX.X, op=ADD
                )
                nc.scalar.activation(rrP[:], sqP[:], AF.Sqrt)
                nc.vector.tensor_scalar(denP[:], sqP[:], 1.0, None, op0=ADD)
                nc.vector.tensor_tensor(denP[:], denP[:], rrP[:], op=MULT)
                nc.vector.reciprocal(recP[:], denP[:])
                nc.vector.tensor_tensor(mmvP[:], sqP[:], recP[:], op=MULT)
                vb2 = mmvP[:].unsqueeze(3).broadcast_to((128, P, NCAPS, OCH))
                nc.vector.tensor_tensor(v16P[:], sx, vb2, op=MULT)
                if it == ITERATIONS - 1:
                    nc.vector.tensor_tensor(v32P[:], sx, vb2, op=MULT)

            def y_pass(bk, uA, it):
                """agreement: y = sum_o u*v, d update, coeff16 = sigmoid."""
                p = bk % 2
                d = d_all[:, bk, :]
                vb = v16P[:, p].unsqueeze(2).broadcast_to((128, NCAPS, CPC, OCH))
                nc.vector.tensor_tensor(wA_v[p], uA[:], vb, op=MULT)
                tree_reduce(p, wA_v[p], OCH, yy[p][:])
                nc.vector.tensor_tensor(dd[p][:], yy[p][:, 0, :], yy[p][:, 1, :], op=SUB)
                if it == 0:
                    nc.vector.tensor_copy(d, dd[p][:])
                else:
                    nc.vector.tensor_tensor(d, d, dd[p][:], op=ADD)
                nc.scalar.activation(co16[p][:, 0, :], d, AF.Sigmoid)
                nc.scalar.activation(co16[p][:, 1, :], d, AF.Sigmoid, scale=-1.0)

            def s_pass(bk, uB):
                """weighted sum: s_partial = sum_c coeff*u -> sp[p]."""
                p = bk % 2
                cb = co16[p][:].unsqueeze(2).broadcast_to((128, NCAPS, OCH, CPC))
                nc.vector.tensor_tensor(wB_v[p], uB[:], cb, op=MULT)
                tree_reduce(p, wB_v[p], CPC, sp[p][:].rearrange("b (n o) -> b n o", n=NCAPS))

            # software pipeline: GEMM emission is pumped in cg-group slices
            # between routing stages so no engine queue serializes a full
            # chunk of GEMM behind a full chunk of routing (in-order queues).
            # When a pair's two generators finish, its iteration-0 AllReduce
            # is emitted right there.
            us = {}
            gens = [gemm_chunk(bk) for bk in range(NBCH)]
            state = {"gi": 0, "done": 0}

            def pump(n):
                while n > 0 and state["gi"] < NBCH:
                    try:
                        next(gens[state["gi"]])
                        state["done"] += 1
                        n -= 1
                    except StopIteration:
                        state["gi"] += 1
                        if state["gi"] % 2 == 0:
                            pj = state["gi"] // 2 - 1
                            cc_pair(pj % 2, [sp0[0][:], sp0[1][:]])

            YIELDS = CPC // CG + 1  # per-chunk generator yields

            def ensure(k):
                if state["done"] < k:
                    pump(k - state["done"])

            ensure(2 * YIELDS)  # prologue: chunks 0 and 1 + pair-0 AllReduce
            for pr in range(NBCH // 2):
                q = pr % 2
                bks = (2 * pr, 2 * pr + 1)
                # this pair's GEMM + iteration-0 AllReduce must be emitted
                # before its routing references them (+1 forces the
                # StopIteration that emits the pair's cc)
                ensure((2 * pr + 2) * YIELDS + 1)
                for it in range(ITERATIONS):
                    # one consolidated pump per iteration, BEFORE the
                    # CC-dependent routing ops: the pumped u-copies sit ahead
                    # of the blocked sqrt/sigmoid in the in-order ACT queue
                    # and execute during the AllReduce flight
                    pump(6)
                    squash_pair(q, it)
                    if it < ITERATIONS - 1:
                        for bk in bks:
                            y_pass(bk, us[bk][0], it)
                            s_pass(bk, us[bk][1])
                        cc_pair(q, [sp[0][:], sp[1][:]])
                for bk in bks:
                    nc.scalar.dma_start(
                        out[bk * BCH : (bk + 1) * BCH, :, :], v32P[:, bk % 2]
                    )

    if split_waits:
        _split_sync_waits(nc)
    return nc


def _prep_inputs(x, W):
    x = np.ascontiguousarray(x, dtype=np.float32)
    W0 = np.ascontiguousarray(W.reshape(NCAPS, C, OCH, ICH), dtype=np.float32)
    xt_cores, wt_cores = [], []
    for k in range(NCORES):
        cs = k * CPC
        xc = x[:, cs : cs + CPC, :]  # (B, 64, 256)
        x6 = xc.reshape(NBCH, BCH, CPC, KH, 128)
        xt = np.ascontiguousarray(x6.transpose(3, 4, 0, 2, 1)).astype(np.float16)
        xt_cores.append(xt)
        Wc = W0[:, cs : cs + CPC]  # (2, 64, 64, 256)
        w5 = Wc.reshape(NCAPS, CPC, OCH, KH, 128)
        wt = np.ascontiguousarray(w5.transpose(3, 4, 1, 0, 2)).reshape(
            KH, 128, CPC, NO
        ).astype(np.float16)
        wt_cores.append(wt)
    return xt_cores, wt_cores


_NC_CACHE = {}


def kernel(x, W):
    global LAST_EXEC_NS
    _install_profile_hook()
    if "nc" not in _NC_CACHE:
        _NC_CACHE["nc"] = build_kernel()
    nc = _NC_CACHE["nc"]
    xts, wts = _prep_inputs(np.asarray(x), np.asarray(W))
    in_maps = [{"xt": xts[k], "wt": wts[k]} for k in range(NCORES)]
    trace = bool(os.environ.get("CAPS_TRACE"))
    res = run_bass_kernel_spmd(nc, in_maps, list(range(NCORES)), trace=trace)
    LAST_EXEC_NS = res.exec_time_ns
    return res.results[0]["out"].astype(np.float32)
